# revision 1
# baseline (speedup 1.0000x reference)
"""Trainium2 Bass kernel for an AttentionBlock (GroupNorm + single-head
spatial self-attention + residual), data-parallel over batch across 8
NeuronCores.  fp8-DoubleRow edition.

Per-sample computation (C=256 channels, N=64*64=4096 positions):
  xn = GroupNorm(x; 8 groups) * gn_w + gn_b
  q = Wq xn + bq ; k = Wk xn + bk ; v = Wv xn + bv
  att = softmax(q^T k / 16)          # [N, N]
  out = v att^T                      # [C, N]
  y = x + Wp out + bp

Key ideas vs the fp32r baseline:
  - q, k, v, attention weights (et), attention output, and Wp all live in
    fp8; every attention matmul runs in DoubleRow perf mode (K=256 packed
    as [128, 2, *] operands, 2x PE throughput).
  - softmax normalization is deferred:  U = v @ et^T,  Z = 1 @ et^T (ones
    DoubleRow matmul accumulated on the PE),  out = U * (1/Z).
  - exp runs with a single shared scale  et = e^{s/16 + LNC}:
      * ACT pairs: native Exp -> e4m3  (max |arg| bounded: e^{smax+LNC}<240)
      * DVE pairs: Schraudolph bit-trick -> u8 = s*TA + TB truncated, bit-
        cast as e5m2 == 2^{(u/4)-15} ~ e^{s/16+LNC}.  u is provably in
        (0, 127) for this problem's score range (s/16 in [-7.69, 7.87]).
    The 1/16 softmax scale and the exp bias live in TA/TB and the ACT
    scale/bias, so q/k stay at natural scale (good for e4m3).
  - QKV projections stay fp32r (precision headroom) and their PSUM drains
    quantize to fp8 on ACT/DVE; xn tiles come from ACT+GPSIMD.
"""

import sys

sys.path.insert(0, "/opt/trn_rl_repo")

import numpy as np
import ml_dtypes

import concourse.bass as bass
import concourse.tile as tile
from concourse import mybir
from concourse.vector_clock import ScopedClock, VectorClock

# ---------------------------------------------------------------------------
# Workaround: this walrus build only accepts 1 sync-wait per instruction, but
# TileContext's final drain attaches one wait per live processor.  Emit one
# drain per processor instead.
# ---------------------------------------------------------------------------


def _patched_drain_and_barrier(self, tick_clock, wait_clock):
    gc = tick_clock.global_clock
    n = len(gc)
    for p in range(n):
        if gc[p] == 0:
            continue
        vec = [0] * n
        vec[p] = gc[p]
        nop = self.nc.sync.nop(nofuse=True, hint="tail_wait")
        wait_clock.add_sem_waits(nop.ins, ScopedClock({None: VectorClock(vec)}))
    self.nc.sync.drain()
    self.nc.all_engine_barrier()
    popped = self.nc._tile_sem_poison_stack.pop()
    assert popped is self._sem_poison
    self.nc.clear_and_free_semaphores(list(self.sems.allocated().values()))
    self.nc.all_engine_barrier()


tile.TileContext._drain_and_barrier = _patched_drain_and_barrier


# ---------------------------------------------------------------------------
# Same 1-wait-per-instruction constraint, applied globally: hoist excess
# sync-waits onto NoOp instructions inserted immediately before the
# over-subscribed one (engines execute their stream in order).
# ---------------------------------------------------------------------------

import json as _json


def _split_excess_waits(bir_bytes: bytes) -> bytes:
    d = _json.loads(bir_bytes)
    changed = False
    for fn in d.get("functions", []):
        for bb in fn.get("blocks", []):
            out = []
            for ins in bb.get("instructions", []):
                si = ins.get("sync_info") or {}
                waits = si.get("on_wait") or []
                if len(waits) > 1 and "engine" in ins:
                    for i, w in enumerate(waits[:-1]):
                        out.append({
                            "engine": ins["engine"],
                            "ins": [],
                            "outs": [],
                            "name": f"{ins['name']}-xw{i}",
                            "opcode": "NoOp",
                            "sync_info": {"on_update": [], "on_wait": [w]},
                            "debug": ins.get("debug", 0),
                        })
                    si["on_wait"] = [waits[-1]]
                    changed = True
                out.append(ins)
            bb["instructions"] = out
    if not changed:
        return bir_bytes
    return _json.dumps(d).encode()


_orig_to_json_bytes = bass.Bass.to_json_bytes


def _patched_to_json_bytes(self):
    return _split_excess_waits(_orig_to_json_bytes(self))


bass.Bass.to_json_bytes = _patched_to_json_bytes

FP32 = mybir.dt.float32
FP32R = mybir.dt.float32r
F8E4 = mybir.dt.float8e4
F8E5 = mybir.dt.float8e5
U8 = mybir.dt.uint8

B = 8          # batch == number of cores
C = 256        # channels
H = W = 64
N = H * W      # 4096 spatial positions
G = 8          # groups
GS = C // G    # 32 channels per group
CB = C // 128  # 2 channel blocks of 128 partitions
IC = 512       # i-chunk width
NI = N // IC   # 8
NJ = N // 128  # 32 j blocks
NJP = NJ // 2  # 16 j pairs
NCH = 512      # n-chunk width for the QKV projections
EPS = 1e-5
INV_CNT = 1.0 / (GS * N)

LOG2E = 1.4426950408889634
LNC = -2.545                      # shared exp scale: et = e^{s/16 + LNC}
TA = 4.0 * LOG2E / 16.0           # Schraudolph multiplier (raw-score input)
TB = 4.0 * (15.0 + LNC * LOG2E)   # HW DVE fp32->u8 convert rounds (verified)
# per-16-pair exp engine assignment: A=ACT native exp (e4m3),
# D=DVE Schraudolph (e5m2).  Alternating 10A/6D and 9A/7D chunks
# balances engine busy time (ACT also carries the q drains, DVE the
# normalize/fin ops).
PATTERNS = [list("AADADADDADAADADA"), list("AADADAADADAADADA")]

DR = mybir.MatmulPerfMode.DoubleRow


def build_bass():
    nc = bass.Bass()

    x_d = nc.declare_dram_parameter("x", [C, N], FP32R, isOutput=False)
    wqT_d = nc.declare_dram_parameter("wqT", [C, C], FP32R, isOutput=False)
    wkT_d = nc.declare_dram_parameter("wkT", [C, C], FP32R, isOutput=False)
    wvT_d = nc.declare_dram_parameter("wvT", [C, C], FP32R, isOutput=False)
    wp8_d = nc.declare_dram_parameter("wp8", [128, 2 * C], F8E4, isOutput=False)
    wpn_d = nc.declare_dram_parameter("wpn", [C, C], FP32R, isOutput=False)
    # packed per-channel consts: [gnw, gnb, bq, bk, bp2, gsel(8)] = 13 -> 16
    cpk_d = nc.declare_dram_parameter("cpk", [C, 16], FP32, isOutput=False)
    bsel_d = nc.declare_dram_parameter("bsel", [G, C], FP32, isOutput=False)
    y_d = nc.declare_dram_parameter("y", [C, N], FP32, isOutput=True)

    Act = mybir.ActivationFunctionType
    Alu = mybir.AluOpType

    with tile.TileContext(nc) as tc:
        with (
            nc.allow_low_precision(reason="fp8/fp32r tensors feeding the PE"),
            tc.tile_pool(name="sb", bufs=1) as sb,
            tc.tile_pool(name="ps", bufs=1, space="PSUM") as ps,
        ):
            # ---------------- load x (critical path: stats wait on it) ----
            # ---- DMA order matters: the SP engine serializes descriptor
            # generation (~0.9us each), so small critical consts go first,
            # then x (stats path), then weights (needed a bit later).
            cpk = [sb.tile([128, 16], FP32, tag=f"cpk{cb}", name=f"cpk{cb}") for cb in range(CB)]
            bsel = sb.tile([G, C], FP32, tag="bsel")

            gnw = [cpk[cb][:, 0:1] for cb in range(CB)]
            gnb = [cpk[cb][:, 1:2] for cb in range(CB)]
            bq = [cpk[cb][:, 2:3] for cb in range(CB)]
            bk = [cpk[cb][:, 3:4] for cb in range(CB)]
            bpc = [cpk[cb][:, 4:5] for cb in range(CB)]
            gsel = [cpk[cb][:, 8:16] for cb in range(CB)]

            # x first (stats are the critical path), split across the SP
            # hardware-DGE queue and the GPSIMD SWDGE queue so the two
            # [128,1024]-piece streams transfer in parallel
            xs = [sb.tile([128, N], FP32R, tag=f"x{cb}", name=f"x{cb}") for cb in range(CB)]
            XH = N // 2
            XQ = N // 4
            for h in range(4):
                for cb in range(CB):
                    eng = nc.sync if (h * CB + cb) % 2 == 0 else nc.gpsimd
                    eng.dma_start(
                        out=xs[cb][:, h * XQ : (h + 1) * XQ],
                        in_=x_d[cb * 128 : (cb + 1) * 128, h * XQ : (h + 1) * XQ],
                    )

            # consts are needed only after the stats (~10us): queue them
            # behind x
            for cb in range(CB):
                sl = slice(cb * 128, (cb + 1) * 128)
                nc.gpsimd.dma_start(out=cpk[cb], in_=cpk_d[sl, :])
            nc.gpsimd.dma_start(out=bsel, in_=bsel_d[:, :])

            # ---------------- weights (behind x on both queues) ------------
            wq = [sb.tile([128, C], FP32R, tag=f"wq{cb}", name=f"wq{cb}") for cb in range(CB)]
            wk = [sb.tile([128, C], FP32R, tag=f"wk{cb}", name=f"wk{cb}") for cb in range(CB)]
            wv = [sb.tile([128, C], FP32R, tag=f"wv{cb}", name=f"wv{cb}") for cb in range(CB)]
            wpn = [sb.tile([128, C], FP32R, tag=f"wpn{cb}", name=f"wpn{cb}") for cb in range(CB)]
            wp8 = sb.tile([128, 2, C], F8E4, tag="wp8", name="wp8")
            for cb in range(CB):
                sl = slice(cb * 128, (cb + 1) * 128)
                nc.sync.dma_start(out=wk[cb], in_=wkT_d[sl, :])
                nc.sync.dma_start(out=wv[cb], in_=wvT_d[sl, :])
                nc.sync.dma_start(out=wq[cb], in_=wqT_d[sl, :])
                nc.sync.dma_start(out=wpn[cb], in_=wpn_d[sl, :])
            nc.sync.dma_start(out=wp8.rearrange("p a b -> p (a b)"), in_=wp8_d[:, :])

            # memset-built constants (DVE producer; no DMA).  fp8/fp32r
            # memsets fail the walrus ISA check, so write the raw bit
            # patterns through integer views (e4m3 1.0 = 0x38).
            ones2u = sb.tile([128, 2, 16], U8, tag="ones2", name="ones2u")
            nc.vector.memset(ones2u.rearrange("p a b -> p (a b)"), 0x38)
            ones2 = ones2u.bitcast(F8E4)
            ones_row = sb.tile([1, 128], FP32R, tag="ones_row", name="ones_row")
            nc.vector.memset(ones_row.bitcast(mybir.dt.uint32), 0x3F800000)
            lnc_t = sb.tile([128, 1], FP32, tag="lnc_t", name="lnc_t")
            nc.vector.memset(lnc_t, LNC)

            # PE observes DMA-queue producers of matmul operands ahead of the
            # real matmuls (1-sync-wait ISA limit workaround).
            def pe_touch(ap):
                nc.tensor.ldweights(ap.bitcast(mybir.dt.bfloat16)[0:1, 0:2])

            for t in (wq + wk + wv + wpn + gsel):
                pe_touch(t)
            pe_touch(wp8.rearrange("p a b -> p (a b)"))
            pe_touch(bsel)

            # PE p-state warm-up: ~3.5us of junk matmuls during the PE's
            # idle head window (weights land ~7.5us, stats not until ~14us)
            # so the k/v projection phase starts at full clock.
            for wi in range(14):
                wps = ps.tile([128, C], FP32, tag="u", bufs=2, name="warmmm")
                nc.tensor.matmul(
                    wps, lhsT=wq[wi % 2][:, 0:128], rhs=wq[(wi + 1) % 2],
                    start=True, stop=True,
                )

            # DVE observes the small-constant DMA queues early.
            for t in (gnw[0], gnw[1], gnb[0], gnb[1]):
                dvt2 = sb.tile([128, 1], FP32, tag="dvt", bufs=1, name="dvt")
                nc.vector.tensor_copy(out=dvt2, in_=t)

            # ---------------- group-norm statistics -----------------------
            stat = [sb.tile([128, 2], FP32, tag=f"stat{cb}", name=f"stat{cb}") for cb in range(CB)]
            SQCH = 1024
            sums = [sb.tile([128, 2], FP32, tag=f"sums{cb}", bufs=1, name="sums") for cb in range(CB)]
            sqas = [sb.tile([128, N // SQCH], FP32, tag=f"sqa{cb}", bufs=1, name="sqa") for cb in range(CB)]
            # stats split: DVE does cb0 sums + cb1 square-folds; ACT does
            # cb0 squares + cb1 sums (Copy+accum); GPSIMD squares cb1.
            for h in range(2):
                nc.vector.reduce_sum(
                    sums[0][:, h : h + 1],
                    xs[0][:, h * XH : (h + 1) * XH],
                    axis=mybir.AxisListType.X,
                )
            for t in range(N // SQCH):
                for cb in range(CB):
                    scr = sb.tile([128, SQCH], FP32, tag="sq_scratch", bufs=4, name="scr")
                    xsl = xs[cb][:, t * SQCH : (t + 1) * SQCH]
                    if cb == 0:
                        nc.scalar.activation(
                            out=scr, in_=xsl, func=Act.Square,
                            accum_out=sqas[cb][:, t : t + 1],
                        )
                    else:
                        # GPSIMD squares + folds (SBUF-only)
                        nc.gpsimd.tensor_mul(out=scr, in0=xsl, in1=xsl)
                        nc.vector.reduce_sum(
                            sqas[cb][:, t : t + 1], scr, axis=mybir.AxisListType.X,
                        )
                if t % 2 == 1:
                    h = t // 2
                    sscr = sb.tile([128, XH], FP32, tag="sum_scratch", bufs=2, name="sscr")
                    nc.scalar.activation(
                        out=sscr, in_=xs[1][:, h * XH : (h + 1) * XH],
                        func=Act.Copy, accum_out=sums[1][:, h : h + 1],
                    )
            for cb in range(CB):
                nc.vector.reduce_sum(stat[cb][:, 0:1], sums[cb], axis=mybir.AxisListType.X)
                nc.vector.reduce_sum(stat[cb][:, 1:2], sqas[cb], axis=mybir.AxisListType.X)

            gstats_ps = ps.tile([G, 2], FP32, tag="u", bufs=2, name="gstats_ps")
            for cb in range(CB):
                nc.tensor.matmul(
                    gstats_ps, lhsT=gsel[cb], rhs=stat[cb],
                    start=(cb == 0), stop=(cb == CB - 1),
                )
            m2 = sb.tile([G, 2], FP32, tag="m2")
            nc.vector.tensor_scalar_mul(out=m2, in0=gstats_ps, scalar1=INV_CNT)
            meansq = sb.tile([G, 1], FP32, tag="meansq")
            nc.vector.tensor_mul(out=meansq, in0=m2[:, 0:1], in1=m2[:, 0:1])
            gm = sb.tile([G, 2], FP32, tag="gm")
            nc.vector.tensor_sub(out=gm[:, 1:2], in0=m2[:, 1:2], in1=meansq)
            eps_t = sb.tile([G, 1], FP32, tag="eps_t")
            nc.vector.memset(eps_t, EPS)
            nc.scalar.activation(out=gm[:, 1:2], in_=gm[:, 1:2], func=Act.Sqrt, bias=eps_t)
            nc.vector.reciprocal(out=gm[:, 1:2], in_=gm[:, 1:2])
            nc.vector.tensor_copy(out=gm[:, 0:1], in_=m2[:, 0:1])

            scale_v = []
            bias_v = []
            for cb in range(CB):
                bvals_ps = ps.tile([128, 2], FP32, tag="u", bufs=2, name="bvals_ps")
                nc.tensor.matmul(
                    bvals_ps, lhsT=bsel[:, cb * 128 : (cb + 1) * 128], rhs=gm,
                    start=True, stop=True,
                )
                sc = sb.tile([128, 1], FP32, tag=f"scale{cb}", name=f"scale{cb}")
                # bias padded to 2 columns: 1-wide moving operands fail the
                # walrus ISA check, so the cascade matmuls read [128, 2]
                bi = sb.tile([128, 2], FP32R, tag=f"bias{cb}", name=f"bias{cb}")
                tmp = sb.tile([128, 1], FP32, tag=f"tmpb{cb}", name=f"tmpb{cb}")
                nc.vector.tensor_mul(out=sc, in0=bvals_ps[:, 1:2], in1=gnw[cb])
                nc.vector.tensor_mul(out=tmp, in0=bvals_ps[:, 0:1], in1=sc)
                nc.vector.memset(bi.bitcast(mybir.dt.uint32), 0)
                nc.vector.tensor_sub(out=bi[:, 0:1], in0=gnb[cb], in1=tmp)
                scale_v.append(sc)
                bias_v.append(bi)
            pe_touch(bias_v[1])

            # ---- fold GroupNorm into the projections:  xn = a*x + b  ------
            # Scaled weights  w2[c, :] = a_c * w[c, :]  turn every projection
            # matmul into one on RAW x; the b-term becomes per-output-channel
            # bias corrections computed by tiny matmuls:
            #   q/k:  gamma_o = sum_c wT[c,o] b_c   (added at the PSUM drain)
            #   v:    beta_o  = sum_c wvT[c,o] b_c  rides through softmax
            #         unchanged (rows sum to 1), so  Wp@beta  joins the fin
            #         bias like bv did.
            wq2 = [sb.tile([128, C], FP32R, tag=f"wq2{cb}", name=f"wq2{cb}") for cb in range(CB)]
            wk2 = [sb.tile([128, C], FP32R, tag=f"wk2{cb}", name=f"wk2{cb}") for cb in range(CB)]
            wv2 = [sb.tile([128, C], FP32R, tag=f"wv2{cb}", name=f"wv2{cb}") for cb in range(CB)]
            for cb in range(CB):
                nc.vector.tensor_scalar_mul(out=wq2[cb], in0=wq[cb], scalar1=scale_v[cb])
                nc.vector.tensor_scalar_mul(out=wk2[cb], in0=wk[cb], scalar1=scale_v[cb])
                nc.vector.tensor_scalar_mul(out=wv2[cb], in0=wv[cb], scalar1=scale_v[cb])

            bq2t, bk2t, bpc2 = [], [], []
            vbeta = []
            for ob in range(CB):
                osl = slice(ob * 128, (ob + 1) * 128)
                for wmat, blist, badd in ((wq, bq2t, bq), (wk, bk2t, bk)):
                    g_ps = ps.tile([128, 2], FP32, tag="u", bufs=2, name="g_ps")
                    for cb in range(CB):
                        nc.tensor.matmul(
                            g_ps, lhsT=wmat[cb][:, osl], rhs=bias_v[cb],
                            start=(cb == 0), stop=(cb == CB - 1),
                        )
                    bt = sb.tile([128, 1], FP32, tag="bqk2", bufs=4, name="bt")
                    nc.vector.tensor_add(out=bt, in0=g_ps[:, 0:1], in1=badd[ob])
                    blist.append(bt)
                vb_ps = ps.tile([128, 2], FP32, tag="u", bufs=2, name="vb_ps")
                for cb in range(CB):
                    nc.tensor.matmul(
                        vb_ps, lhsT=wv[cb][:, osl], rhs=bias_v[cb],
                        start=(cb == 0), stop=(cb == CB - 1),
                    )
                vb = sb.tile([128, 2], FP32R, tag="vbeta", bufs=2, name="vb")
                nc.vector.tensor_copy(out=vb, in_=vb_ps)
                vbeta.append(vb)
            for ob in range(CB):
                osl = slice(ob * 128, (ob + 1) * 128)
                wb_ps = ps.tile([128, 2], FP32, tag="u", bufs=2, name="wb_ps")
                for cb in range(CB):
                    nc.tensor.matmul(
                        wb_ps, lhsT=wpn[cb][:, osl], rhs=vbeta[cb],
                        start=(cb == 0), stop=(cb == CB - 1),
                    )
                bp3 = sb.tile([128, 1], FP32, tag="bp3", bufs=2, name="bp3")
                nc.vector.tensor_add(out=bp3, in0=wb_ps[:, 0:1], in1=bpc[ob])
                bpc2.append(bp3)

            # ---------------- QKV projections (fp32r MMs, fp8 outputs) ----
            # q8/k8: [c_lo(128), c_hi(2), n]; v8: [j_lo(128), j_hi(2), jp*C+c]
            q8 = sb.tile([128, 2, N], F8E4, tag="q8", name="q8")
            k8 = sb.tile([128, 2, N], F8E4, tag="k8", name="k8")
            v8 = sb.tile([128, 2, NJP * C], F8E4, tag="v8", name="v8")

            # k / v projections straight from raw x with GN-folded weights.
            # Emitted chunk-by-chunk interleaved into attention chunk 0's
            # pair loop: pair jp only needs k/v blocks from chunk jp//2, so
            # the PE-bound projection work overlaps attention-0's exp work.
            def emit_kv(nch):
                nsl = slice(nch * NCH, (nch + 1) * NCH)
                # k: [o_block, nch] = sum_cb wk2[cb][:, ob]^T @ x[cb]
                for ob in range(CB):
                    osl = slice(ob * 128, (ob + 1) * 128)
                    mm = ps.tile([128, IC], FP32, tag="st", bufs=5, name="mmqk")
                    for cb in range(CB):
                        nc.tensor.matmul(
                            mm, lhsT=(wk2[cb][:, osl]), rhs=(xs[cb][:, nsl]),
                            start=(cb == 0), stop=(cb == CB - 1),
                        )
                    if ob == 0:
                        nc.scalar.activation(
                            out=k8[:, ob : ob + 1, nsl], in_=mm,
                            func=Act.Identity, bias=bk2t[ob], scale=1.0,
                        )
                    else:
                        nc.vector.tensor_scalar_add(
                            out=k8[:, ob : ob + 1, nsl], in0=mm, scalar1=bk2t[ob],
                        )
                # v: per 128-wide n block: vT[j, c] = x[:, jb]^T @ wv2
                for nb in range(NCH // 128):
                    jb = nch * (NCH // 128) + nb
                    jp, hh = jb // 2, jb % 2
                    bsl = slice(nch * NCH + nb * 128, nch * NCH + (nb + 1) * 128)
                    stv = ps.tile([128, IC], FP32, tag="st", bufs=5, name="mmv")
                    mmv = stv[:, 0:C]
                    for cb in range(CB):
                        # v's beta and bv both ride through softmax into fin
                        nc.tensor.matmul(
                            mmv, lhsT=(xs[cb][:, bsl]), rhs=(wv2[cb]),
                            start=(cb == 0), stop=(cb == CB - 1),
                        )
                    vdst = v8[:, hh : hh + 1, jp * C : (jp + 1) * C]
                    if nb < 2:
                        nc.scalar.copy(out=vdst, in_=mmv)
                    else:
                        nc.vector.tensor_copy(out=vdst, in_=mmv)

            # ---------------- attention (per i-chunk) ---------------------
            # Chunks of i-columns; the final 512 are split into two 256-wide
            # sub-chunks so the dangling serial normalize/proj tail at the
            # very end of the kernel is half-sized.
            CHUNKS = [(ci * IC, IC) for ci in range(NI)]

            # Each chunk's normalize/proj tail is DEFERRED into the next
            # chunk's pair stream so the serial zr->zb->out8->proj chain
            # never stalls the in-order PE/ACT/DVE queues at a boundary.
            def make_tails(u_ps, z_ps, i0, w, last=False):
                # PSUM handles allocated NOW (ring-FIFO order), ops emitted
                # later from inside the next chunk.
                isl = slice(i0, i0 + w)
                zb_ps = ps.tile([128, IC], FP32, tag="z", bufs=1, name="zb")
                pps = [ps.tile([128, IC], FP32, tag="st", bufs=5, name="pp")
                       for _ in range(CB)]
                zr = sb.tile([1, IC], FP32R, tag="zr", bufs=3, name="zr")
                nc.vector.reciprocal(out=zr[:, 0:w], in_=z_ps[:, 0:w])

                out8 = sb.tile([128, 2, IC], F8E4, tag="out8", bufs=3, name="out8")

                def tail1():
                    # normalize: out8 = U * (1/Z) broadcast across partitions
                    # (zb must bounce through SBUF: engines may read only one
                    # non-scalar input from PSUM per instruction)
                    nc.tensor.matmul(
                        zb_ps[:, 0:w], lhsT=ones_row, rhs=zr[:, 0:w],
                        start=True, stop=True,
                    )
                    zbs = sb.tile([128, IC], FP32, tag="zb", bufs=3, name="zbs")
                    nc.vector.tensor_copy(out=zbs[:, 0:w], in_=zb_ps[:, 0:w])
                    for cb in range(CB):
                        nc.vector.tensor_mul(
                            out=out8[:, cb : cb + 1, 0:w], in0=u_ps[cb][:, 0:w],
                            in1=zbs[:, 0:w],
                        )

                def tail2():
                    # projection (DoubleRow) + bias + residual
                    for ob in range(CB):
                        osl = slice(ob * 128, (ob + 1) * 128)
                        nc.tensor.matmul(
                            pps[ob][:, 0:w], lhsT=wp8[:, :, osl],
                            rhs=out8[:, :, 0:w],
                            start=True, stop=True, perf_mode=DR,
                        )
                        fin = sb.tile([128, IC], FP32, tag="fin", bufs=4, name="fin")
                        nc.vector.scalar_tensor_tensor(
                            out=fin[:, 0:w], in0=pps[ob][:, 0:w], scalar=bpc2[ob],
                            in1=xs[ob][:, isl],
                            op0=Alu.add, op1=Alu.add,
                        )
                        if last and ob == 1:
                            # final write: ACT's DGE queue is idle at the
                            # kernel tail, run the two y transfers in parallel
                            nc.scalar.dma_start(out=y_d[osl, isl], in_=fin[:, 0:w])
                        else:
                            nc.sync.dma_start(out=y_d[osl, isl], in_=fin[:, 0:w])

                return tail1, tail2

            def emit_qproj(i0, w):
                # q projection for chunk at [i0, i0+w), straight from raw x
                qisl = slice(i0, i0 + w)
                for ob in range(CB):
                    osl = slice(ob * 128, (ob + 1) * 128)
                    mm = ps.tile([128, IC], FP32, tag="st", bufs=5, name="mmq")
                    for cb in range(CB):
                        nc.tensor.matmul(
                            mm[:, 0:w], lhsT=(wq2[cb][:, osl]),
                            rhs=(xs[cb][:, qisl]),
                            start=(cb == 0), stop=(cb == CB - 1),
                        )
                    nc.scalar.activation(
                        out=q8[:, ob : ob + 1, qisl], in_=mm[:, 0:w],
                        func=Act.Identity, bias=bq2t[ob], scale=1.0,
                    )

            # Flat software pipeline across chunk boundaries: the AV/Z for
            # pair jp issues after the scores of the NEXT pair, and that
            # skew carries across chunks (the next chunk's first scores run
            # ahead of this chunk's final AV), so the PE never drains at a
            # boundary.
            prev_tails = None
            pend = None
            pend_mk = None
            emit_kv(0)
            emit_qproj(*CHUNKS[0])
            for ci, (i0, w) in enumerate(CHUNKS):
                q_rhs = q8[:, :, i0 : i0 + w]

                u_ps = [
                    ps.tile([128, IC], FP32, tag="u", bufs=2, name=f"u{cb}_{ci}")
                    for cb in range(CB)
                ]
                z_ps = ps.tile([1, IC], FP32, tag="z", bufs=1, name=f"z{ci}")

                def emit_av(jp, av_rhs, u_ps=u_ps, z_ps=z_ps, w=w):
                    if jp == NJP - 1:
                        # Z first: zr (and the whole normalize tail) hangs
                        # off the final Z matmul
                        nc.tensor.matmul(
                            z_ps[:, 0:w], lhsT=ones2[:, :, 0:1], rhs=av_rhs,
                            start=False, stop=True, perf_mode=DR,
                        )
                    for cb in range(CB):
                        nc.tensor.matmul(
                            u_ps[cb][:, 0:w],
                            lhsT=v8[:, :, jp * C + cb * 128 : jp * C + (cb + 1) * 128],
                            rhs=av_rhs,
                            start=(jp == 0), stop=(jp == NJP - 1), perf_mode=DR,
                        )
                    if jp != NJP - 1:
                        nc.tensor.matmul(
                            z_ps[:, 0:w], lhsT=ones2[:, :, 0:1], rhs=av_rhs,
                            start=(jp == 0), stop=False, perf_mode=DR,
                        )

                PATTERN = PATTERNS[0 if ci == 0 else 1]
                for jp in range(NJP):
                    if ci == 0 and jp >= 1 and jp % 2 == 1 and jp < 15:
                        emit_kv((jp + 1) // 2)
                    if PATTERN[jp] == "A":
                        et = sb.tile([128, 2, IC], F8E4, tag="et4", bufs=4, name="et4")
                        av_rhs = et[:, :, 0:w]
                    else:
                        et = sb.tile([128, 2, IC], U8, tag="et5", bufs=4, name="et5")
                        av_rhs = et.bitcast(F8E5)[:, :, 0:w]
                    for hh in range(2):
                        jb = 2 * jp + hh
                        jsl = slice(jb * 128, (jb + 1) * 128)
                        st = ps.tile([128, IC], FP32, tag="st", bufs=5, name="st")
                        nc.tensor.matmul(
                            st[:, 0:w], lhsT=k8[:, :, jsl], rhs=q_rhs,
                            start=True, stop=True, perf_mode=DR,
                        )
                        if PATTERN[jp] == "A":
                            nc.scalar.activation(
                                out=et[:, hh : hh + 1, 0:w], in_=st[:, 0:w],
                                func=Act.Exp,
                                bias=lnc_t, scale=1.0 / 16.0,
                            )
                        else:
                            nc.vector.tensor_scalar(
                                out=et[:, hh : hh + 1, 0:w], in0=st[:, 0:w],
                                scalar1=TA, scalar2=TB,
                                op0=Alu.mult, op1=Alu.add,
                            )
                    # previous chunk's deferred tail ops go in FRONT of this
                    # chunk's first AV matmuls (avoids a circular queue wait)
                    if prev_tails is not None:
                        if jp == 1:
                            prev_tails[0]()
                        elif jp == 3:
                            prev_tails[1]()
                            prev_tails = None
                    # next chunk's q projection: early enough that its
                    # drains land before that chunk's first score matmul
                    if jp == NJP - 1 and ci + 1 < len(CHUNKS):
                        emit_qproj(*CHUNKS[ci + 1])
                    if pend is not None:
                        pfn, pjp, prhs = pend
                        pfn(pjp, prhs)
                        if pjp == NJP - 1:
                            # previous chunk fully accumulated: emit its zr
                            # and hand its tails to this chunk
                            prev_tails = pend_mk()
                    pend = (emit_av, jp, av_rhs)
                pend_mk = (lambda u_ps=u_ps, z_ps=z_ps, i0=i0, w=w,
                           last=(ci == len(CHUNKS) - 1):
                           make_tails(u_ps, z_ps, i0, w, last))

            # flush: final chunk's last AV, then its tail (nothing to hide in)
            pfn, pjp, prhs = pend
            pfn(pjp, prhs)
            last_tails = pend_mk()
            last_tails[0]()
            last_tails[1]()

    return nc


def _prep_inputs(x_full, gn_w, gn_b, wq, bq, wk, bk, wv, bv, wp, bp):
    """Host-side input prep shared by all cores (weights) + per-core x."""
    f = np.float32
    wqT = np.ascontiguousarray(wq.T.astype(f))
    wkT = np.ascontiguousarray(wk.T.astype(f))
    wvT = np.ascontiguousarray(wv.T.astype(f))
    wpn = np.ascontiguousarray(wp.T.astype(f))
    # wp8[p, i, o] = wp[o, p + 128*i]  (lhsT for the DoubleRow proj matmul)
    wp8 = np.ascontiguousarray(
        wp.T.astype(f).reshape(2, 128, C).transpose(1, 0, 2).reshape(128, 2 * C)
    ).astype(ml_dtypes.float8_e4m3)
    bp2 = (np.asarray(bp, np.float64)
           + np.asarray(wp, np.float64) @ np.asarray(bv, np.float64)
           ).astype(f)
    gsel = np.zeros((C, G), f)
    for c in range(C):
        gsel[c, c // GS] = 1.0
    bsel = np.ascontiguousarray(gsel.T)
    # packed per-channel consts: [gnw, gnb, bq, bk, bp2, 0, 0, 0, gsel(8)]
    cpk = np.zeros((C, 16), f)
    cpk[:, 0] = gn_w.astype(f)
    cpk[:, 1] = gn_b.astype(f)
    cpk[:, 2] = bq.astype(f)
    cpk[:, 3] = bk.astype(f)
    cpk[:, 4] = bp2
    cpk[:, 8:16] = gsel
    shared = dict(
        wqT=wqT, wkT=wkT, wvT=wvT, wpn=wpn, wp8=wp8, cpk=cpk, bsel=bsel,
    )
    in_maps = []
    for b in range(B):
        m = dict(shared)
        m["x"] = np.ascontiguousarray(x_full[b].reshape(C, N).astype(f))
        in_maps.append(m)
    return in_maps


_CACHED_NC = None


def _get_nc():
    global _CACHED_NC
    if _CACHED_NC is None:
        _CACHED_NC = build_bass()
    return _CACHED_NC


def kernel(x, gn_w, gn_b, wq, bq, wk, bk, wv, bv, wp, bp):
    from concourse.bass_utils import run_bass_kernel_spmd

    x = np.asarray(x)
    in_maps = _prep_inputs(
        np.asarray(x), np.asarray(gn_w), np.asarray(gn_b),
        np.asarray(wq), np.asarray(bq), np.asarray(wk), np.asarray(bk),
        np.asarray(wv), np.asarray(bv), np.asarray(wp), np.asarray(bp),
    )
    nc = _get_nc()
    res = run_bass_kernel_spmd(nc, in_maps, list(range(B)))
    out = np.empty((B, C, H, W), np.float32)
    for b in range(B):
        out[b] = res.results[b]["y"].reshape(C, H, W)
    return out

